# revision 8
# baseline (speedup 1.0000x reference)
"""Trainium2 Bass kernel for CompositionModel (gnn_message_passing), v2.

Model: per-cell MLP over [log1p(X) ++ Z[cell_to_batch]] followed by a
segment-mean over batch labels.

Strategy (all-fp8 device pipeline, host reduce):
  * Host: precompute log1p(X) and quantize everything to fp8 e4m3.
    Per 512-cell block the device sees one [128, 1024] fp8 tile:
    cols 0-511 = Xs = fp8(log1p(X)/8) (features on partitions), cols
    512-1023 a "Z-pack" k-tile whose rows carry Zc_hi, Zc_residual,
    constant rows for an exact (hi + lo/16) b1, Z-weight-correction
    rows, and a 4-block rotation of X-weight-correction rows. With
    weights quantized as hi + scaled-lo pairs, layer 1 is a SINGLE
    DoubleRow matmul per 128-wide output half - corrections included.
  * Layer 2 = fp8 DoubleRow vs W2SCALE-scaled fp8 W2 (hi every block,
    pre-scaled lo every LO_EVERY-th block, first-order exact through
    the relu + segment mean).
  * Device per block: 2 DR matmuls -> ACT relu (fp8 h1) -> 2(+2) DR
    matmuls -> DVE cast (raw fp8 ps2) -> SWDGE store. No bias, no
    relu2, no reduction on device.
  * Host epilogue: decode fp8 ps2, add b2, relu, apply W3/b3 and the
    segment mean in f32 numpy.
"""

import numpy as np
import ml_dtypes

import concourse.bacc as bacc
import concourse.mybir as mybir
import concourse.tile as tile
from concourse.bass_utils import run_bass_kernel_spmd

BF16 = ml_dtypes.bfloat16
FP8 = ml_dtypes.float8_e4m3fn

N_CORES = 8
DX = 128
DZ = 32
H = 256
BLK = 512          # cells per block
NBLK = 124         # blocks per core for the 500k-cell reference input
LO_EVERY = 2       # W2 lo-correction applied every LO_EVERY-th block
W2SCALE = 64.0     # fp8 pre-scale on W2, divided out on the host

_compiled = {}
_last_in_maps = None


def _build_program(nblk):
    f32 = mybir.dt.float32
    fp8 = mybir.dt.float8e4
    Act = mybir.ActivationFunctionType
    DR = mybir.MatmulPerfMode.DoubleRow
    nsb = nblk // 2

    nc = bacc.Bacc("TRN2", target_bir_lowering=False, debug=False,
                   num_devices=N_CORES)

    in_d = nc.dram_tensor("inp", [nsb, 128, 2048], fp8, kind="ExternalInput")
    w1_d = nc.dram_tensor("w1", [4, 2, 128, 256], fp8, kind="ExternalInput")
    w2_d = nc.dram_tensor("w2", [2, 2, 128, 256], fp8, kind="ExternalInput")
    out_d = nc.dram_tensor("out", [nsb, 128, 2048], fp8, kind="ExternalOutput")

    with tile.TileContext(nc) as tc:
        with tc.tile_pool(name="consts", bufs=1) as cpool, \
             tc.tile_pool(name="work", bufs=6) as pool, \
             tc.tile_pool(name="psum", bufs=2, space="PSUM") as psum:

            # weight preamble on the scalar HWDGE queue so it overlaps the
            # first input-tile loads on the sync queue
            w1t = {}
            for q in range(4):
                for h in range(2):
                    w = cpool.tile([128, 256], fp8, tag=f"w1_{q}{h}")
                    nc.scalar.dma_start(w[:], w1_d[q, h])
                    w1t[q, h] = w[:].rearrange("p (k m) -> p k m", k=2)
            w2t = {}
            for h in range(2):
                for t in range(2):
                    w = cpool.tile([128, 256], fp8, tag=f"w2_{h}{t}")
                    nc.scalar.dma_start(w[:], w2_d[h, t])
                    w2t[h, t] = w[:].rearrange("p (k m) -> p k m", k=2)

            # Block-granular software pipeline with a 4-block skew. Each
            # iteration emits: L2 matmuls for block b-4, L1 matmuls for
            # block b, the ps2 cast for b-4 and the relu for b. Every
            # buffer dependency (ps1/ps2 reuse needs the consumer two
            # blocks back to be done; h1 is consumed four blocks later)
            # has >= 2 block-periods of slack, so no engine waits on a
            # stage issued in the same iteration.
            SKEW = 4
            it_of = {}
            h1_of = {}
            ob_of = {}
            for b in range(nblk + SKEW):
                c = b - SKEW
                if b < nblk and b % 2 == 0:
                    it = pool.tile([128, 2048], fp8, tag="in")
                    nc.sync.dma_start(it[:], in_d[b // 2])
                    it_of[b // 2] = it
                if c >= 0:
                    h1v = h1_of.pop(c)[:].rearrange("p (k c) -> p k c", k=2)
                    lo = c % LO_EVERY == 0
                    ps2 = psum.tile([128, 1024], f32, tag="ps2")
                    nc.tensor.matmul(ps2[:, 0:512], w2t[0, 0], h1v,
                                     start=True, stop=not lo, perf_mode=DR)
                    if lo:
                        nc.tensor.matmul(ps2[:, 0:512], w2t[0, 1], h1v,
                                         start=False, stop=True, perf_mode=DR)
                    nc.tensor.matmul(ps2[:, 512:1024], w2t[1, 0], h1v,
                                     start=True, stop=not lo, perf_mode=DR)
                    if lo:
                        nc.tensor.matmul(ps2[:, 512:1024], w2t[1, 1], h1v,
                                         start=False, stop=True, perf_mode=DR)
                if b < nblk:
                    it = it_of[b // 2]
                    hb = b % 2
                    qc = b % 4
                    xv = it[:, hb * 1024:(hb + 1) * 1024].rearrange(
                        "p (k c) -> p k c", k=2)
                    ps1 = psum.tile([128, 1024], f32, tag="ps1")
                    nc.tensor.matmul(ps1[:, 0:512], w1t[qc, 0], xv,
                                     start=True, stop=True, perf_mode=DR)
                    nc.tensor.matmul(ps1[:, 512:1024], w1t[qc, 1], xv,
                                     start=True, stop=True, perf_mode=DR)
                if c >= 0:
                    if c % 2 == 0:
                        ob = pool.tile([128, 2048], fp8, tag="ob")
                        ob_of[c // 2] = ob
                    ob = ob_of[c // 2]
                    nc.vector.tensor_copy(
                        ob[:, (c % 2) * 1024:(c % 2 + 1) * 1024], ps2[:])
                    if c % 2 == 1:
                        nc.gpsimd.dma_start(out_d[c // 2],
                                            ob_of.pop(c // 2)[:])
                if b < nblk:
                    h1 = pool.tile([128, 1024], fp8, tag="h1")
                    nc.scalar.activation(h1[:], ps1[:], Act.Relu)
                    h1_of[b] = h1

    nc.compile()
    return nc


def _get_program(nblk):
    if nblk not in _compiled:
        _compiled[nblk] = _build_program(nblk)
    return _compiled[nblk]


def _q(x):
    return np.asarray(x, np.float32).astype(FP8)


def kernel(X, Z, W1, b1, W2, b2, W3, b3, cell_to_batch, sample_idx_batch):
    X = np.asarray(X, dtype=np.float32)
    Z = np.asarray(Z, dtype=np.float32)
    W1 = np.asarray(W1, dtype=np.float32)
    b1 = np.asarray(b1, dtype=np.float32)
    W2 = np.asarray(W2, dtype=np.float32)
    b2 = np.asarray(b2, dtype=np.float32)
    W3 = np.asarray(W3, dtype=np.float32)
    b3 = np.asarray(b3, dtype=np.float32)
    c2b = np.asarray(cell_to_batch).astype(np.int64)
    sib = np.asarray(sample_idx_batch).astype(np.int64)

    n = X.shape[0]
    nseg = sib.shape[0]
    seg = sib[c2b]
    d_out = W3.shape[1]

    per_core = -(-n // N_CORES)                     # cells per core (unpadded)
    nblk = max(4, 4 * (-(-per_core // (4 * BLK))))  # multiple-of-4 blocks so
    # per-core slices stay aligned with the global 4-block q-class rotation
    ncap = nblk * BLK                               # padded cells per core
    nsb = nblk // 2

    # ---- quantized weights -------------------------------------------------
    W1x = W1[0:DX]
    W1z = W1[DX:DX + DZ]
    A1 = 8.0 * W1x
    W1x_hi = _q(A1)
    D = A1 - W1x_hi.astype(np.float32)
    W1x_corr16 = _q(16.0 * D)
    W1z_hi = _q(W1z)
    W1z_r = _q(W1z / 2.0)
    W1z_lo16 = _q(16.0 * (W1z - W1z_hi.astype(np.float32)))
    W1z_lo64 = _q(64.0 * (W1z - W1z_hi.astype(np.float32)))
    b1_hi = _q(b1)
    b1_lo16 = _q(16.0 * (b1 - b1_hi.astype(np.float32)))

    # w1 stationary tiles [q, half, p, k*128+m]
    w1q = np.zeros((4, 2, 128, 256), dtype=FP8)
    zr_base = np.zeros((128, H), dtype=FP8)
    zr_base[0:32] = W1z_hi
    zr_base[32:64] = W1z_r
    zr_base[64] = b1_hi
    zr_base[65] = b1_lo16
    zr_base[66:96] = W1z_lo16[0:30]
    for q in range(4):
        zr = zr_base.copy()
        if q < 3:
            zr[96:128] = W1x_corr16[32 * q:32 * q + 32]
        else:
            zr[96:126] = W1x_corr16[96:126]
            zr[126:128] = W1z_lo64[30:32]
        for h in range(2):
            w1q[q, h, :, 0:128] = W1x_hi[:, h * 128:(h + 1) * 128]
            w1q[q, h, :, 128:256] = zr[:, h * 128:(h + 1) * 128]

    # w2 stationary tiles [half, term, p, k*128+m] (k = h1 row-block)
    A2 = W2SCALE * W2
    W2_hi = _q(A2)
    W2_lo = _q(LO_EVERY * (A2 - W2_hi.astype(np.float32)))
    w2q = np.zeros((2, 2, 128, 256), dtype=FP8)
    for t, term in enumerate((W2_hi, W2_lo)):
        km = term.reshape(2, 128, H).transpose(1, 0, 2)  # [p, k, m]
        for h in range(2):
            w2q[h, t] = km[:, :, h * 128:(h + 1) * 128].reshape(128, 256)

    # ---- per-cell quantized activations ------------------------------------
    lx = np.log1p(X)
    Xs = _q(lx / 8.0)                  # [n, 128]
    Xs32 = _q(lx / 32.0)
    Z_hi = _q(Z)
    Z_r = _q(2.0 * (Z - Z_hi.astype(np.float32)))
    Z_hi16 = _q(Z_hi.astype(np.float32) / 16.0)
    Zc_hi = Z_hi[c2b]
    Zc_r = Z_r[c2b]
    Zc_hi16 = Z_hi16[c2b]

    zp = np.zeros((n, 128), dtype=FP8)
    zp[:, 0:32] = Zc_hi
    zp[:, 32:64] = Zc_r
    zp[:, 64] = FP8(1.0)
    zp[:, 65] = FP8(0.0625)
    zp[:, 66:96] = Zc_hi16[:, 0:30]
    qcls = (np.arange(n) // BLK) % 4   # block class before core split
    # rows 96-127: rotating X-weight corrections (q<3) / leftovers (q==3)
    for q in range(3):
        m = qcls == q
        zp[m, 96:128] = Xs32[m][:, 32 * q:32 * q + 32]
    m = qcls == 3
    zp[m, 96:126] = Xs32[m][:, 96:126]
    zp[m, 126:128] = Zc_hi16[m][:, 30:32]

    # NOTE: block classes are computed on the GLOBAL cell index, and cores get
    # contiguous slices of ncap cells, so per-core block boundaries align with
    # the global ones only if per-core offsets are multiples of 4*BLK. ncap is
    # nblk*BLK with nblk even; ensure nblk % 4 == 0 so q-classes stay aligned.

    # ---- assemble per-core input blocks ------------------------------------
    in_arr = np.zeros((N_CORES, nsb, 128, 2048), dtype=FP8)
    for c in range(N_CORES):
        s, e = c * ncap, min(n, (c + 1) * ncap)
        if s >= e:
            continue
        cnt = e - s
        xs_c = np.zeros((ncap, 128), dtype=FP8)
        xs_c[:cnt] = Xs[s:e]
        zp_c = np.zeros((ncap, 128), dtype=FP8)
        zp_c[:cnt] = zp[s:e]
        xs_r = xs_c.reshape(nsb, 2, BLK, 128)
        zp_r = zp_c.reshape(nsb, 2, BLK, 128)
        for hb in range(2):
            in_arr[c, :, :, hb * 1024:hb * 1024 + 512] = \
                xs_r[:, hb].transpose(0, 2, 1)
            in_arr[c, :, :, hb * 1024 + 512:(hb + 1) * 1024] = \
                zp_r[:, hb].transpose(0, 2, 1)

    # ---- run on 8 cores ----------------------------------------------------
    nc = _get_program(nblk)
    in_maps = []
    for c in range(N_CORES):
        in_maps.append({"inp": in_arr[c], "w1": w1q, "w2": w2q})
    global _last_in_maps
    _last_in_maps = in_maps
    res = run_bass_kernel_spmd(nc, in_maps, list(range(N_CORES)))

    # ---- host epilogue -----------------------------------------------------
    sums = np.zeros((nseg, d_out), np.float32)
    lut = np.arange(256, dtype=np.uint8).view(FP8).astype(np.float32)
    for c in range(N_CORES):
        s, e = c * ncap, min(n, (c + 1) * ncap)
        if s >= e:
            continue
        cnt = e - s
        o = res.results[c]["out"]                      # [nsb, 128, 2048] fp8
        o = lut[o.view(np.uint8)]
        o = o.reshape(nsb, 128, 2, 2, BLK)             # [sb, p, hb, half, c]
        o = o.transpose(0, 2, 4, 3, 1).reshape(ncap, H)[:cnt]
        h2 = np.maximum(o / W2SCALE + b2[None, :], 0.0)
        y16 = h2 @ W3
        seg_c = seg[s:e]
        for j in range(d_out):
            sums[:, j] += np.bincount(seg_c, weights=y16[:, j],
                                      minlength=nseg)

    counts = np.bincount(seg, minlength=nseg).astype(np.float32)
    Y = sums / np.maximum(counts, 1.0)[:, None] + b3[None, :]
    Y[counts == 0] = 0.0
    return Y.astype(np.float32)
